# revision 12
# baseline (speedup 1.0000x reference)
"""Trainium2 Bass kernel for nn_Decoder_58265526338342 (GNN decoder).

Math (reference, with A = sparse adjacency from (edge_row <- edge_col, w)):
    Ax  = A @ x
    h   = relu(Ax @ W1 + b1)
    Ah  = A @ h                      (only rows in idx needed)
    pred_attr = Ah[idx] @ W2 + b2
    s   = Ax[idx] @ W_sd + b_sd
    pred_adj  = s @ s.T

Distribution over 8 NeuronCores (one chip):
  - node i owned by core i % 8 (2500 nodes/core, 20 windows of 128)
  - phase 1: edges partitioned by dest owner; segment-sum as PE matmuls
    with one-hot S matrices built on-device (S = (iota==row_local)*w),
    sources fetched by dma_gather from a replicated x
  - phase 2 source-sharded: each core aggregates A@h partials from its
    LOCAL h over sel-slot windows, one ReduceScatter(sum) distributes
    Ah[idx] chunks
  - s: computed from local Ax rows, AllGather -> every core holds full s
    for its block-row of s @ s.T
"""
import numpy as np

import concourse.bass as bass
import concourse.bacc as bacc
import concourse.mybir as mybir
import concourse.tile as tile
from concourse.bass_utils import run_bass_kernel_spmd

M = 8            # cores
N = 20000        # nodes
E = 320000       # edges
F = 256          # feature dim
K = 4096         # selected nodes
WIN = 128        # dest window size (PSUM partitions)
LOCN = N // M    # 2500
NW1 = (LOCN + WIN - 1) // WIN        # 20
LOCNP = NW1 * WIN                    # 2560
F32 = mybir.dt.float32
I16 = mybir.dt.int16


def _cdiv(a, b):
    return (a + b - 1) // b


def _wrap_idx(flat_idx, nblk, blk):
    """[nblk*blk] int -> [128, nblk*blk/16] int16 dma_gather layout."""
    w = flat_idx.reshape(nblk, blk // 16, 16).transpose(0, 2, 1)  # [nblk,16,blk/16]
    w = np.swapaxes(w, 0, 1).reshape(16, nblk * (blk // 16))
    return np.tile(w, (8, 1)).astype(np.int16)


def _host_prep(edge_row, edge_col, edge_w, labels):
    edge_row = np.asarray(edge_row).astype(np.int64)
    edge_col = np.asarray(edge_col).astype(np.int64)
    edge_w = np.asarray(edge_w).astype(np.float32)
    labels = np.asarray(labels)

    # selected indices per jnp.nonzero(labels==1, size=K)[0] semantics
    idx = np.nonzero(labels == 1)[0]
    if len(idx) >= K:
        idx = idx[:K]
    else:
        idx = np.concatenate([idx, np.zeros(K - len(idx), np.int64)])

    # ---------- phase-1 edges: dest-owner partition, windows of local dest
    own = edge_row % M
    ldst = edge_row // M
    win = ldst // WIN
    w1cnt = np.zeros((M, NW1), np.int64)
    p1 = [[None] * NW1 for _ in range(M)]
    for m in range(M):
        s = own == m
        ld, cols, ws, wn = ldst[s], edge_col[s], edge_w[s], win[s]
        order = np.argsort(wn, kind="stable")
        ld, cols, ws, wn = ld[order], cols[order], ws[order], wn[order]
        bounds = np.searchsorted(wn, np.arange(NW1 + 1))
        for w in range(NW1):
            a, b = bounds[w], bounds[w + 1]
            p1[m][w] = (cols[a:b], ld[a:b] - w * WIN, ws[a:b])
            w1cnt[m, w] = b - a
    B1 = max(1, int(_cdiv(w1cnt.max(), 128)))

    g1 = np.zeros((M, NW1 * B1 * 128), np.int64)
    seg1 = np.zeros((M, NW1 * B1 * 128), np.float32)
    wt1 = np.zeros((M, NW1 * B1 * 128), np.float32)
    for m in range(M):
        for w in range(NW1):
            cols, segs, ws = p1[m][w]
            o, n = w * B1 * 128, len(cols)
            g1[m, o:o + n] = cols
            seg1[m, o:o + n] = segs
            wt1[m, o:o + n] = ws

    # ---------- sel assignment (owner-based, padded uniform)
    sel_pos = [np.nonzero((idx % M) == m)[0] for m in range(M)]
    SELC = max(128, _cdiv(max(len(s) for s in sel_pos), 128) * 128)
    SELTOT = M * SELC
    NW2 = SELTOT // WIN
    sel_local = np.zeros((M, SELC), np.int64)
    row_of = np.zeros(K, np.int64)
    for m in range(M):
        p = sel_pos[m]
        sel_local[m, :len(p)] = idx[p] // M
        row_of[p] = m * SELC + np.arange(len(p))

    # ---------- phase-2 edges: source-sharded, dest = global sel slot
    node_slots = {}
    for p in range(K):
        node_slots.setdefault(int(idx[p]), []).append(int(row_of[p]))
    in_sel = np.isin(edge_row, idx)
    e2r, e2c, e2w = edge_row[in_sel], edge_col[in_sel], edge_w[in_sel]
    slots_l, cols_l, ws_l = [], [], []
    for r, c, wv in zip(e2r, e2c, e2w):
        for s in node_slots[int(r)]:
            slots_l.append(s)
            cols_l.append(c)
            ws_l.append(wv)
    slots_a = np.array(slots_l, np.int64)
    cols_a = np.array(cols_l, np.int64)
    ws_a = np.array(ws_l, np.float32)
    own2 = cols_a % M
    w2cnt = np.zeros((M, NW2), np.int64)
    p2 = [[None] * NW2 for _ in range(M)]
    for m in range(M):
        s = own2 == m
        sl, cl, wl = slots_a[s], cols_a[s], ws_a[s]
        wn = sl // WIN
        order = np.argsort(wn, kind="stable")
        sl, cl, wl, wn = sl[order], cl[order], wl[order], wn[order]
        bounds = np.searchsorted(wn, np.arange(NW2 + 1))
        for w in range(NW2):
            a, b = bounds[w], bounds[w + 1]
            lc = cl[a:b] // M
            p2[m][w] = ((lc % 128) * NW1 + lc // 128, sl[a:b] - w * WIN, wl[a:b])
            w2cnt[m, w] = b - a
    B2 = max(1, int(_cdiv(w2cnt.max(), 128)))
    g2 = np.zeros((M, NW2 * B2 * 128), np.int64)
    seg2 = np.zeros((M, NW2 * B2 * 128), np.float32)
    wt2 = np.zeros((M, NW2 * B2 * 128), np.float32)
    for m in range(M):
        for w in range(NW2):
            gidx, segs, wl = p2[m][w]
            o, n = w * B2 * 128, len(gidx)
            g2[m, o:o + n] = gidx
            seg2[m, o:o + n] = segs
            wt2[m, o:o + n] = wl

    # sel gather indices into local ax layout rows (p*NW1+w)
    selg = np.zeros((M, SELC), np.int64)
    for m in range(M):
        l = sel_local[m]
        selg[m] = (l % 128) * NW1 + l // 128

    return dict(g1=g1, seg1=seg1, wt1=wt1, B1=B1,
                g2=g2, seg2=seg2, wt2=wt2, B2=B2,
                selg=selg, SELC=SELC, SELTOT=SELTOT, NW2=NW2,
                row_of=row_of)


STAGE = 99   # bisect knob: 1=phase1 agg, 2=+hT/h, 3=+s/AG, 4=+phase2/RS, 5=+adj, 99=all


def _build_program(nc, B1, B2, SELC):
    SELTOT = M * SELC
    NW2 = SELTOT // WIN
    NSEL = SELC // 128            # sel row-chunks per core
    CT = _cdiv(SELC, 512)         # 512-col tiles within SELC
    rg = [list(range(M))]

    x_d = nc.dram_tensor("x", [N, F], F32, kind="ExternalInput")
    g1_d = nc.dram_tensor("g1", [128, NW1 * B1 * 8], I16, kind="ExternalInput")
    s1_d = nc.dram_tensor("s1", [128, NW1 * B1], F32, kind="ExternalInput")
    w1e_d = nc.dram_tensor("w1e", [128, NW1 * B1], F32, kind="ExternalInput")
    g2_d = nc.dram_tensor("g2", [128, NW2 * B2 * 8], I16, kind="ExternalInput")
    s2_d = nc.dram_tensor("s2", [128, NW2 * B2], F32, kind="ExternalInput")
    w2e_d = nc.dram_tensor("w2e", [128, NW2 * B2], F32, kind="ExternalInput")
    sg_d = nc.dram_tensor("sg", [128, SELC // 16], I16, kind="ExternalInput")
    iota_d = nc.dram_tensor("iota", [128, 128], F32, kind="ExternalInput")
    ident_d = nc.dram_tensor("ident", [128, 128], F32, kind="ExternalInput")
    wm_d = nc.dram_tensor("wm", [3, 128, 2, F], F32, kind="ExternalInput")  # W1,W2,Wsd
    bias_d = nc.dram_tensor("bias", [128, 3, 2], F32, kind="ExternalInput")  # b1,b2,bsd

    attr_d = nc.dram_tensor("attrT", [128, 2 * SELC], F32, kind="ExternalOutput")
    adj_d = nc.dram_tensor("adj", [SELC, SELTOT], F32, kind="ExternalOutput")

    ie = mybir.AluOpType.is_equal
    mu = mybir.AluOpType.mult
    ad = mybir.AluOpType.add
    Relu = mybir.ActivationFunctionType.Relu

    with tile.TileContext(nc) as tc:
        with (
            tc.tile_pool(name="const", bufs=1) as cp,
            tc.tile_pool(name="dram", bufs=1, space="DRAM") as dp,
            tc.tile_pool(name="psA", bufs=2, space="PSUM") as psA,   # [128,256] agg
            tc.tile_pool(name="psT", bufs=2, space="PSUM") as psT,   # [128,128] transpose
            tc.tile_pool(name="psB", bufs=2, space="PSUM") as psB,   # [128,512] dense
            tc.tile_pool(name="st", bufs=4) as stp,
            tc.tile_pool(name="big", bufs=1) as bigp,
        ):
            # ---- constants / aux loads
            g1_sb = cp.tile([128, NW1 * B1 * 8], I16)
            s1_sb = cp.tile([128, NW1 * B1], F32)
            w1e_sb = cp.tile([128, NW1 * B1], F32)
            g2_sb = cp.tile([128, NW2 * B2 * 8], I16)
            s2_sb = cp.tile([128, NW2 * B2], F32)
            w2e_sb = cp.tile([128, NW2 * B2], F32)
            sg_sb = cp.tile([128, SELC // 16], I16)
            iota_sb = cp.tile([128, 128], F32)
            ident_sb = cp.tile([128, 128], F32)
            wm_sb = cp.tile([128, 3, 2, F], F32)
            bias_sb = cp.tile([128, 3, 2], F32)
            nc.sync.dma_start(g1_sb[:], g1_d[:])
            nc.sync.dma_start(s1_sb[:], s1_d[:])
            nc.sync.dma_start(w1e_sb[:], w1e_d[:])
            nc.sync.dma_start(g2_sb[:], g2_d[:])
            nc.sync.dma_start(s2_sb[:], s2_d[:])
            nc.sync.dma_start(w2e_sb[:], w2e_d[:])
            nc.sync.dma_start(sg_sb[:], sg_d[:])
            nc.sync.dma_start(iota_sb[:], iota_d[:])
            nc.sync.dma_start(ident_sb[:], ident_d[:])
            nc.sync.dma_start(wm_sb[:], wm_d[:].rearrange("m p k f -> p m k f"))
            nc.sync.dma_start(bias_sb[:], bias_d[:])

            ax_d = dp.tile([LOCNP, F], F32)      # local Ax, row = p*NW1+w
            h_d = dp.tile([LOCNP, F], F32)       # local h,  row = p*NW1+w
            ahp_d = dp.tile([SELTOT, F], F32)    # Ah partials, slot-major
            ahm_d = dp.tile([SELC, F], F32)      # my Ah chunk (RS out)
            sT_in_d = dp.tile([128, 2 * SELC], F32)
            sT_all_d = dp.tile([M, 128, 2 * SELC], F32, addr_space="Shared")

            ax_dv = ax_d[:].rearrange("(p w) f -> p w f", p=128)
            h_dv = h_d[:].rearrange("(p w) f -> p w f", p=128)
            ahp_dv = ahp_d[:].rearrange("(w p) f -> p w f", p=128)

            # ================= phase 1: Ax = A @ x =================
            axT_sb = bigp.tile([128, 2, LOCNP], F32)
            with (
                tc.tile_pool(name="xg", bufs=2) as xgp,
                tc.tile_pool(name="stage", bufs=3) as sgp,
            ):
                GCH = 4          # batches per dma_gather call (<=512 idxs)
                for w in range(NW1):
                    xg = xgp.tile([128, B1, F], F32, tag="xg")
                    for g0 in range(0, B1, GCH):
                        gb = min(GCH, B1 - g0)
                        nc.gpsimd.dma_gather(
                            xg[:, g0:g0 + gb, :], x_d[:],
                            g1_sb[:, (w * B1 + g0) * 8:(w * B1 + g0 + gb) * 8],
                            num_idxs=gb * 128, num_idxs_reg=gb * 128, elem_size=F)
                    acc = psA.tile([128, F], F32, tag="acc")
                    for b in range(B1):
                        st = stp.tile([128, 128], F32, tag="st")
                        c = w * B1 + b
                        nc.vector.tensor_scalar(
                            st[:], iota_sb[:], s1_sb[:, c:c + 1], w1e_sb[:, c:c + 1],
                            op0=ie, op1=mu)
                        nc.tensor.matmul(acc[:], st[:], xg[:, b, :],
                                         start=(b == 0), stop=(b == B1 - 1))
                    axst = sgp.tile([128, F], F32, tag="axst")
                    nc.vector.tensor_copy(axst[:], acc[:])
                    nc.sync.dma_start(ax_dv[:, w, :], axst[:])
                    for c in range(2):
                        pt = psT.tile([128, 128], F32, tag="pt")
                        nc.tensor.transpose(pt[:], axst[:, c * 128:(c + 1) * 128],
                                            ident_sb[:])
                        nc.vector.tensor_copy(axT_sb[:, c, w * 128:(w + 1) * 128],
                                              pt[:])

            if STAGE < 2:
                return nc
            # ---- hT = relu(W1.T @ axT + b1) ; h node-major -> h_d per window
            with tc.tile_pool(name="hstage", bufs=3) as hsp:
                for nt in range(LOCNP // 512):
                    hTst = hsp.tile([128, 2, 512], F32, tag="hTst")
                    for c in range(2):
                        ph = psB.tile([128, 512], F32, tag="ph")
                        for k in range(2):
                            nc.tensor.matmul(
                                ph[:], wm_sb[:, 0, k, c * 128:(c + 1) * 128],
                                axT_sb[:, k, nt * 512:(nt + 1) * 512],
                                start=(k == 0), stop=(k == 1))
                        nc.scalar.activation(hTst[:, c, :], ph[:],
                                             Relu, bias=bias_sb[:, 0, c:c + 1])
                    for wl in range(4):       # 4 node windows of 128 inside nt
                        w = nt * 4 + wl
                        hst = hsp.tile([128, F], F32, tag="hst")
                        for c in range(2):
                            pt = psT.tile([128, 128], F32, tag="pt")
                            nc.tensor.transpose(
                                pt[:], hTst[:, c, wl * 128:(wl + 1) * 128],
                                ident_sb[:])
                            nc.vector.tensor_copy(hst[:, c * 128:(c + 1) * 128],
                                                  pt[:])
                        nc.sync.dma_start(h_dv[:, w, :], hst[:])

            if STAGE < 3:
                return nc
            # ---- s chunk: gather Ax[sel], sT = Wsd.T @ axgT + bsd; AllGather
            axg = bigp.tile([128, NSEL, F], F32)
            nc.gpsimd.dma_gather(axg[:], ax_d[:], sg_sb[:],
                                 num_idxs=SELC, num_idxs_reg=SELC, elem_size=F)
            axsT = bigp.tile([128, 2, SELC], F32)
            for j in range(NSEL):
                for c in range(2):
                    pt = psT.tile([128, 128], F32, tag="pt")
                    nc.tensor.transpose(pt[:], axg[:, j, c * 128:(c + 1) * 128],
                                        ident_sb[:])
                    nc.vector.tensor_copy(axsT[:, c, j * 128:(j + 1) * 128], pt[:])
            sT_sb = bigp.tile([128, 2, SELC], F32)
            for c in range(2):
                for t in range(CT):
                    lo, hi = t * 512, min((t + 1) * 512, SELC)
                    ps = psB.tile([128, 512], F32, tag="ph")
                    for k in range(2):
                        nc.tensor.matmul(
                            ps[:, :hi - lo], wm_sb[:, 2, k, c * 128:(c + 1) * 128],
                            axsT[:, k, lo:hi], start=(k == 0), stop=(k == 1))
                    nc.vector.tensor_scalar_add(
                        sT_sb[:, c, lo:hi], ps[:, :hi - lo],
                        bias_sb[:, 2, c:c + 1])
            nc.sync.dma_start(sT_in_d[:], sT_sb[:])
            nc.gpsimd.collective_compute(
                "AllGather", mybir.AluOpType.bypass, replica_groups=rg,
                ins=[sT_in_d[:]], outs=[sT_all_d[:]])

            if STAGE < 4:
                return nc
            # ================= phase 2: Ah partials =================
            with (
                tc.tile_pool(name="hg", bufs=2) as hgp,
                tc.tile_pool(name="ahst", bufs=3) as ahsp,
            ):
                for w in range(NW2):
                    hg = hgp.tile([128, B2, F], F32, tag="hg")
                    for g0 in range(0, B2, 4):
                        gb = min(4, B2 - g0)
                        nc.gpsimd.dma_gather(
                            hg[:, g0:g0 + gb, :], h_d[:],
                            g2_sb[:, (w * B2 + g0) * 8:(w * B2 + g0 + gb) * 8],
                            num_idxs=gb * 128, num_idxs_reg=gb * 128, elem_size=F)
                    acc = psA.tile([128, F], F32, tag="acc")
                    for b in range(B2):
                        st = stp.tile([128, 128], F32, tag="st")
                        c = w * B2 + b
                        nc.vector.tensor_scalar(
                            st[:], iota_sb[:], s2_sb[:, c:c + 1], w2e_sb[:, c:c + 1],
                            op0=ie, op1=mu)
                        nc.tensor.matmul(acc[:], st[:], hg[:, b, :],
                                         start=(b == 0), stop=(b == B2 - 1))
                    ahst = ahsp.tile([128, F], F32, tag="ahst")
                    nc.vector.tensor_copy(ahst[:], acc[:])
                    nc.sync.dma_start(ahp_dv[:, w, :], ahst[:])
            nc.gpsimd.collective_compute(
                "ReduceScatter", mybir.AluOpType.add, replica_groups=rg,
                ins=[ahp_d[:]], outs=[ahm_d[:]])

            if STAGE < 5:
                return nc
            # ================= s @ s.T =================
            sall_sb = bigp.tile([128, M, 2, SELC], F32)
            nc.sync.dma_start(
                sall_sb[:].rearrange("p r k s -> p r (k s)"),
                sT_all_d[:].rearrange("r p x -> p r x"))
            with tc.tile_pool(name="adjst", bufs=4) as adjp:
                for rc in range(NSEL):
                    for r in range(M):
                        for t in range(CT):
                            lo, hi = t * 512, min((t + 1) * 512, SELC)
                            pa = psB.tile([128, 512], F32, tag="ph")
                            for k in range(2):
                                nc.tensor.matmul(
                                    pa[:, :hi - lo],
                                    sT_sb[:, k, rc * 128:(rc + 1) * 128],
                                    sall_sb[:, r, k, lo:hi],
                                    start=(k == 0), stop=(k == 1))
                            ao = adjp.tile([128, 512], F32, tag="ao")
                            nc.vector.tensor_copy(ao[:, :hi - lo], pa[:, :hi - lo])
                            nc.sync.dma_start(
                                adj_d[rc * 128:(rc + 1) * 128,
                                      r * SELC + lo:r * SELC + hi],
                                ao[:, :hi - lo])

            if STAGE < 6:
                return nc
            # ================= pred_attr =================
            ahm_sb = bigp.tile([128, NSEL, F], F32)
            nc.sync.dma_start(ahm_sb[:],
                              ahm_d[:].rearrange("(t p) f -> p t f", p=128))
            ahT = bigp.tile([128, 2, SELC], F32)
            for j in range(NSEL):
                for c in range(2):
                    pt = psT.tile([128, 128], F32, tag="pt")
                    nc.tensor.transpose(pt[:], ahm_sb[:, j, c * 128:(c + 1) * 128],
                                        ident_sb[:])
                    nc.vector.tensor_copy(ahT[:, c, j * 128:(j + 1) * 128], pt[:])
            atT = bigp.tile([128, 2, SELC], F32)
            for c in range(2):
                for t in range(CT):
                    lo, hi = t * 512, min((t + 1) * 512, SELC)
                    ps = psB.tile([128, 512], F32, tag="ph")
                    for k in range(2):
                        nc.tensor.matmul(
                            ps[:, :hi - lo], wm_sb[:, 1, k, c * 128:(c + 1) * 128],
                            ahT[:, k, lo:hi], start=(k == 0), stop=(k == 1))
                    nc.vector.tensor_scalar_add(
                        atT[:, c, lo:hi], ps[:, :hi - lo],
                        bias_sb[:, 1, c:c + 1])
            nc.sync.dma_start(attr_d[:], atT[:].rearrange("p k s -> p (k s)"))
    return nc


_CACHE = {}


def _prepare(feature_attribute, edge_row, edge_col, edge_w, labels,
             W1_ad, b1_ad, W2_ad, b2_ad, W_sd, b_sd):
    x = np.ascontiguousarray(np.asarray(feature_attribute, np.float32))
    prep = _host_prep(edge_row, edge_col, edge_w, labels)
    B1, B2, SELC = prep["B1"], prep["B2"], prep["SELC"]

    key = (B1, B2, SELC, STAGE)
    if key not in _CACHE:
        nc = bacc.Bacc(None, target_bir_lowering=False, debug=False, num_devices=M)
        _build_program(nc, B1, B2, SELC)
        nc.compile()
        _CACHE[key] = nc
    nc = _CACHE[key]

    iota = np.tile(np.arange(128, dtype=np.float32), (128, 1))
    ident = np.eye(128, dtype=np.float32)
    wm = np.stack([
        np.asarray(W1_ad, np.float32).reshape(2, 128, F),
        np.asarray(W2_ad, np.float32).reshape(2, 128, F),
        np.asarray(W_sd, np.float32).reshape(2, 128, F),
    ]).transpose(0, 2, 1, 3)                       # [3, 128, 2, F]
    bias = np.stack([
        np.asarray(b1_ad, np.float32).reshape(2, 128).T,
        np.asarray(b2_ad, np.float32).reshape(2, 128).T,
        np.asarray(b_sd, np.float32).reshape(2, 128).T,
    ], 1)                                          # [128, 3, 2]

    in_maps = []
    for m in range(M):
        in_maps.append(dict(
            x=x,
            g1=_wrap_idx(prep["g1"][m], NW1 * B1, 128),
            s1=prep["seg1"][m].reshape(NW1 * B1, 128).T.copy(),
            w1e=prep["wt1"][m].reshape(NW1 * B1, 128).T.copy(),
            g2=_wrap_idx(prep["g2"][m], prep["NW2"] * B2, 128),
            s2=prep["seg2"][m].reshape(prep["NW2"] * B2, 128).T.copy(),
            w2e=prep["wt2"][m].reshape(prep["NW2"] * B2, 128).T.copy(),
            sg=_wrap_idx(prep["selg"][m], SELC // 128, 128),
            iota=iota, ident=ident, wm=wm, bias=bias,
        ))
    return nc, in_maps, prep


def _assemble(outs, prep):
    SELC = prep["SELC"]
    attr_cat = np.concatenate(
        [outs[m]["attrT"].reshape(128, 2, SELC).transpose(2, 1, 0).reshape(SELC, F)
         for m in range(M)], 0)                    # [SELTOT, F]
    adj_cat = np.concatenate([outs[m]["adj"] for m in range(M)], 0)  # [SELTOT, SELTOT]
    r = prep["row_of"]
    pred_attr = np.ascontiguousarray(attr_cat[r])
    pred_adj = np.ascontiguousarray(adj_cat[r][:, r])
    return pred_attr, pred_adj


def kernel(feature_attribute, edge_row, edge_col, edge_w, labels,
           W1_ad, b1_ad, W2_ad, b2_ad, W_sd, b_sd):
    nc, in_maps, prep = _prepare(feature_attribute, edge_row, edge_col, edge_w,
                                 labels, W1_ad, b1_ad, W2_ad, b2_ad, W_sd, b_sd)
    res = run_bass_kernel_spmd(nc, in_maps, core_ids=list(range(M)))
    return _assemble(res.results, prep)


def bench(np_inputs, iters=5):
    """Time device execution (inputs pre-staged on device, no donation).

    Returns (best_seconds, (pred_attr, pred_adj)).
    """
    import time
    import jax
    from jax.experimental.shard_map import shard_map
    from jax.sharding import Mesh, PartitionSpec, NamedSharding
    from concourse import bass2jax

    nc, in_maps, prep = _prepare(**np_inputs)
    bass2jax.install_neuronx_cc_hook()

    in_names, out_names, out_avals, zero_outs = [], [], [], []
    partition_name = nc.partition_id_tensor.name if nc.partition_id_tensor else None
    for alloc in nc.m.functions[0].allocations:
        if not isinstance(alloc, mybir.MemoryLocationSet):
            continue
        name = alloc.memorylocations[0].name
        if alloc.kind == "ExternalInput":
            if name != partition_name:
                in_names.append(name)
        elif alloc.kind == "ExternalOutput":
            shape = tuple(alloc.tensor_shape)
            dt = mybir.dt.np(alloc.dtype)
            out_names.append(name)
            out_avals.append(jax.core.ShapedArray(shape, dt))
            zero_outs.append(np.zeros(shape, dt))
    n_params = len(in_names)
    all_in_names = list(in_names) + list(out_names)
    if partition_name is not None:
        all_in_names.append(partition_name)

    def _body(*args):
        operands = list(args)
        if partition_name is not None:
            operands.append(bass2jax.partition_id_tensor())
        outs = bass2jax._bass_exec_p.bind(
            *operands, out_avals=tuple(out_avals), in_names=tuple(all_in_names),
            out_names=tuple(out_names), lowering_input_output_aliases=(),
            sim_require_finite=True, sim_require_nnan=True, nc=nc)
        return tuple(outs)

    devices = jax.devices()[:M]
    mesh = Mesh(np.asarray(devices), ("core",))
    nin = n_params + len(out_names)
    sharded = jax.jit(
        shard_map(_body, mesh=mesh, in_specs=(PartitionSpec("core"),) * nin,
                  out_specs=(PartitionSpec("core"),) * len(out_names),
                  check_rep=False),
        keep_unused=True)
    sh = NamedSharding(mesh, PartitionSpec("core"))
    staged = [
        jax.device_put(
            np.concatenate([np.asarray(in_maps[c][nm]) for c in range(M)], 0), sh)
        for nm in in_names
    ]
    staged += [
        jax.device_put(np.zeros((M * z.shape[0], *z.shape[1:]), z.dtype), sh)
        for z in zero_outs
    ]
    out_arrs = jax.block_until_ready(sharded(*staged))   # compile + warm
    best = float("inf")
    for _ in range(iters):
        t0 = time.perf_counter()
        out_arrs = jax.block_until_ready(sharded(*staged))
        best = min(best, time.perf_counter() - t0)
    outs = [
        {nm: np.asarray(out_arrs[i]).reshape(M, *out_avals[i].shape)[c]
         for i, nm in enumerate(out_names)}
        for c in range(M)
    ]
    return best, _assemble(outs, prep)
